# revision 3
# baseline (speedup 1.0000x reference)
"""Multi-head attention (B=2, S=2048, H=1024, 16 heads x 64) on 8 trn2 cores.

Sharding: data-parallel over batch (2) x tensor-parallel over heads (4 groups
of 4 heads). Core c handles batch c//4, head-group c%4 (wq/wk/wv columns
[256*g, 256*g+256)). Host slices inputs per core (pre-cast to bf16 and
pre-blocked into the [quarter, partition, kb, seq] layout the SBUF tiles
use) and concatenates the per-core head-slice outputs.

Per-core schedule (bf16 matmuls, fp32 PSUM accumulation). The kernel is
ACT(exp)-bound in steady state, so everything is organized to start the
exp stream early and keep it dense:

  - inputs arrive as S/4 "quarter" blocks ([128, 8kb, 512] per tensor),
    DMA-prioritized so the first score tile's deps (wq, q0, wk, k0) land
    first; later quarters stream in behind the compute.
  - scores are computed transposed, ST[keys, q-512], as row-tiled pairs
    (two heads on PE row groups (0,0)/(64,0) run concurrently); one
    [128, 1024] PSUM tile per key-tile feeds a single 1024-col exp
    (scale=1/32; no max subtraction - logits are O(0.25) by construction).
    The score pipeline runs TWO groups ahead of the exp stream so filler
    chains never starve the ACT engine.
  - V is projected directly into the transposed [keys, d] layout the PV
    matmuls need: per key-tile, an 8-chunk chain with lhsT = v-chunk and
    rhs = wv (both head-pairs at once, FD=258), plus a ones-row x
    (bias|1|bias) rank-1 matmul that adds the bias AND writes the shared
    ones column - no PE transposes, no per-head copies.
  - PV accumulates out'^T [65, 512] over the 16 key tiles; the shared
    ones column gives the softmax denominator as row 64/0 for free.
  - finalize: PE-transpose out'^T back to [q, 65] (f32), reciprocal of
    the denominator, per-row scale, stage 4 heads, DMA out (f32).
  - projection/finalize work is drip-fed into the PE slack of the exp
    stream as deadline-tagged fillers (projection chains split into
    4-matmul halves so no single filler exceeds the exp-stream buffer);
    segments run m-major so the m=1 projections spread across the m=0
    segments.

The softmax mask of the reference is a mathematical no-op (it broadcasts
over the key axis, shifting every logit of a row equally), so it is ignored.
"""

import numpy as np

B, S, H = 2, 2048, 1024
NH, D = 16, 64            # heads, head_dim
CORES = 8
GROUP_COLS = 256          # 4 heads per core
SCALE = 1.0 / 32.0        # 1/sqrt(H)

_CACHE = {}


def _build():
    import concourse.bacc as bacc
    import concourse.tile as tile
    import concourse.mybir as mybir
    from concourse.masks import make_identity
    from contextlib import ExitStack

    F32 = mybir.dt.float32
    BF16 = mybir.dt.bfloat16
    EXP = mybir.ActivationFunctionType.Exp

    nc = bacc.Bacc("TRN2", target_bir_lowering=False, debug=False,
                   num_devices=CORES)

    NS = S // 128          # 16 key tiles
    NK = H // 128          # 8 contraction tiles over H
    NQ = S // 512          # 4 q-tiles / quarters of 512
    NM = 2                 # head-pairs per core
    VW = 2 * 129           # vh row: [A|1|B] per head-pair

    # blocked inputs: [quarter, partition, kb, seq-in-quarter]
    q_d = nc.dram_tensor("q", [NQ, 128, NK, 512], BF16,
                         kind="ExternalInput").ap()
    k_d = nc.dram_tensor("k", [NQ, 128, NK, 512], BF16,
                         kind="ExternalInput").ap()
    v_d = nc.dram_tensor("v", [NQ, 128, NK, 512], BF16,
                         kind="ExternalInput").ap()
    wq_d = nc.dram_tensor("wq", [128, NK, GROUP_COLS], BF16,
                          kind="ExternalInput").ap()
    wk_d = nc.dram_tensor("wk", [128, NK, GROUP_COLS], BF16,
                          kind="ExternalInput").ap()
    wv_d = nc.dram_tensor("wv", [128, NK, VW], BF16,
                          kind="ExternalInput").ap()
    bq_d = nc.dram_tensor("bq", [GROUP_COLS, 1], F32,
                          kind="ExternalInput").ap()
    bk_d = nc.dram_tensor("bk", [GROUP_COLS, 1], F32,
                          kind="ExternalInput").ap()
    bv_d = nc.dram_tensor("bv", [1, VW], BF16, kind="ExternalInput").ap()
    out_d = nc.dram_tensor("out", [S, GROUP_COLS], F32,
                           kind="ExternalOutput").ap()

    with tile.TileContext(nc) as tc, ExitStack() as es:
        const = es.enter_context(tc.tile_pool(name="const", bufs=1))
        xpool = es.enter_context(tc.tile_pool(name="x", bufs=1))
        wpool = es.enter_context(tc.tile_pool(name="w", bufs=1))
        proj = es.enter_context(tc.tile_pool(name="proj", bufs=1))
        vhp = es.enter_context(tc.tile_pool(name="vh", bufs=1))
        pexpp = es.enter_context(tc.tile_pool(name="pexp", bufs=6))
        pvsbp = es.enter_context(tc.tile_pool(name="pvsb", bufs=4))
        stagep = es.enter_context(tc.tile_pool(name="stage", bufs=1))
        recp = es.enter_context(tc.tile_pool(name="rec", bufs=8))
        # PSUM: st = [128,1024] x3 slots (6 banks; also serves projection
        # accumulators and finalize transposes); pva/pvb = 2 banks.
        ps_st = es.enter_context(tc.tile_pool(name="ps_st", bufs=3,
                                              space="PSUM"))
        ps_pv = es.enter_context(tc.tile_pool(name="ps_pv", bufs=1,
                                              space="PSUM"))

        ident = const.tile([128, 128], F32, tag="ident")
        make_identity(nc, ident[:])
        ones_row = const.tile([1, 128], BF16, tag="ones_row")
        nc.vector.memset(ones_row[:], 1.0)

        # ---- DMAs in priority order (SP issues in emission order) ----
        wqb = wpool.tile([128, NK, GROUP_COLS], BF16, tag="wqb")
        nc.sync.dma_start(out=wqb[:], in_=wq_d[:, :, :])
        xq = xpool.tile([128, NQ, NK, 512], BF16, tag="xq")
        nc.sync.dma_start(out=xq[:, 0], in_=q_d[0])
        wkb = wpool.tile([128, NK, GROUP_COLS], BF16, tag="wkb")
        nc.sync.dma_start(out=wkb[:], in_=wk_d[:, :, :])
        xk = xpool.tile([128, NQ, NK, 512], BF16, tag="xk")
        nc.sync.dma_start(out=xk[:, 0], in_=k_d[0])
        wvb = wpool.tile([128, NK, VW], BF16, tag="wvb")
        nc.sync.dma_start(out=wvb[:], in_=wv_d[:, :, :])
        bvt = const.tile([1, VW], BF16, tag="bvt")
        nc.sync.dma_start(out=bvt[:], in_=bv_d[:, :])
        bias_t = {}
        for x, b_d in (("q", bq_d), ("k", bk_d)):
            bt = const.tile([128, NM], F32, tag=f"b{x}", name=f"b{x}t")
            nc.sync.dma_start(
                out=bt[:], in_=b_d.rearrange("(m p) o -> p m o", p=128)
                .rearrange("p m o -> p (m o)"))
            for m in range(NM):
                bias_t[(x, m)] = bt[:, m:m + 1]
        xv = xpool.tile([128, NQ, NK, 512], BF16, tag="xv")
        nc.sync.dma_start(out=xv[:, 0], in_=v_d[0])
        for i in range(1, NQ):
            nc.sync.dma_start(out=xk[:, i], in_=k_d[i])
            nc.sync.dma_start(out=xv[:, i], in_=v_d[i])
            nc.sync.dma_start(out=xq[:, i], in_=q_d[i])

        # persistent projection outputs
        QT = [proj.tile([128, S], BF16, tag=f"qt{m}", name=f"QT{m}")
              for m in range(NM)]
        KT = [proj.tile([128, S], BF16, tag=f"kt{m}", name=f"KT{m}")
              for m in range(NM)]
        VH = vhp.tile([128, NS, VW], BF16, tag="vh")

        wbf = {"q": wqb, "k": wkb}
        xbf = {"q": xq, "k": xk}

        def proj_qk_half(x, m, nt, half, st):
            # half 0: kb 0..3 into a fresh acc; half 1: kb 4..7 + evacuate.
            if half == 0:
                acc = ps_st.tile([128, 1024], F32, tag="st", name="acc")
                st["acc"] = acc
            acc = st["acc"]
            a = acc[:, 0:512]
            for kb in range(4 * half, 4 * half + 4):
                nc.tensor.matmul(
                    a, wbf[x][:, kb, 128 * m:128 * m + 128],
                    xbf[x][:, nt, kb, :],
                    start=(kb == 0), stop=(kb == NK - 1))
            if half == 1:
                dst = (QT if x == "q" else KT)[m][:, 512 * nt:512 * nt + 512]
                nc.vector.tensor_scalar_add(dst, a, bias_t[(x, m)])

        def proj_qk_nt(x, m, nt):
            st = {}
            proj_qk_half(x, m, nt, 0, st)
            proj_qk_half(x, m, nt, 1, st)

        def proj_vh(s):
            # VH[s] rows = [vh_A | 1 | vh_B] for key rows 128s..128s+128,
            # both head-pairs (cols [0,129) pair0, [129,258) pair1).
            nt, sub = divmod(s, 4)
            acc = ps_st.tile([128, 1024], F32, tag="st", name="acc")
            a = acc[:, 0:VW]
            for kb in range(NK):
                nc.tensor.matmul(
                    a, xv[:, nt, kb, 128 * sub:128 * sub + 128],
                    wvb[:, kb, :], start=(kb == 0), stop=False)
            # rank-1: adds bias everywhere and writes the ones column
            nc.tensor.matmul(a, ones_row[:, 0:128], bvt[:, :],
                             start=False, stop=True)
            nc.vector.tensor_copy(VH[:, s, :], a)

        stages = {qt: stagep.tile([128, 4, GROUP_COLS], F32, tag=f"stage{qt}",
                                  name=f"stage{qt}") for qt in range(NQ)}

        # ---- attention pipeline with deadline-driven PE fillers ----
        # segment = (m, qt); group = key tile kt (both heads, 1024 cols)
        NG = NS
        segs = [{"qt": qt, "m": m, "pva": None, "pvb": None, "idx": 4 * m + qt}
                for m in range(NM) for qt in range(NQ)]

        def half_filler(x, m, nt, dl):
            st = {}
            return [(dl, lambda: proj_qk_half(x, m, nt, 0, st)),
                    ((dl[0], dl[1] + 1), lambda: proj_qk_half(x, m, nt, 1, st))]

        fq = []
        for s in range(NS):                      # VH just-in-time in seg 0
            fq.append(((0, max(0, s - 1)), (lambda s_=s: proj_vh(s_))))
        fq += half_filler("k", 0, 1, (0, 0))
        fq += half_filler("k", 0, 2, (0, 4))
        fq += half_filler("k", 0, 3, (0, 8))
        fq += half_filler("q", 0, 1, (0, 12))    # QT[0] for seg 1
        fq += half_filler("k", 1, 0, (1, 3))
        fq += half_filler("k", 1, 1, (1, 8))
        fq += half_filler("q", 0, 2, (1, 12))    # QT[0] for seg 2
        fq += half_filler("k", 1, 2, (2, 3))
        fq += half_filler("k", 1, 3, (2, 8))
        fq += half_filler("q", 0, 3, (2, 12))    # QT[0] for seg 3
        fq += half_filler("q", 1, 0, (3, 7))     # QT[1] for seg 4
        fq += half_filler("q", 1, 1, (4, 12))    # QT[1] for seg 5
        fq += half_filler("q", 1, 2, (5, 12))
        fq += half_filler("q", 1, 3, (6, 12))
        fq.sort(key=lambda fd: fd[0])

        def pump(upto):
            while fq and fq[0][0] <= upto:
                fq.pop(0)[1]()

        def emit_scores(seg, kt):
            qt, m = seg["qt"], seg["m"]
            stt = ps_st.tile([128, 1024], F32, tag="st", name="stt")
            for a in (0, 1):
                p0 = 64 * a
                nc.tensor.matmul(
                    stt[:, 512 * a:512 * a + 512],
                    KT[m][p0:p0 + 64, 128 * kt:128 * kt + 128],
                    QT[m][p0:p0 + 64, 512 * qt:512 * qt + 512],
                    start=True, stop=True, tile_position=(p0, 0))
            pe = pexpp.tile([128, 1024], BF16, tag="pexp", name="pexp")
            nc.scalar.activation(pe[:], stt[:], EXP, scale=SCALE)
            return pe

        def emit_pv(seg, kt, pe):
            m = seg["m"]
            if seg["pva"] is None:
                seg["pva"] = ps_pv.tile([65, 512], F32, tag="pva", name="pva")
                seg["pvb"] = ps_pv.tile([65, 512], F32, tag="pvb", name="pvb")
            for a in (0, 1):
                pv = seg["pva"] if a == 0 else seg["pvb"]
                lo = 129 * m + 64 * a
                nc.tensor.matmul(pv[:], VH[:, kt, lo:lo + 65],
                                 pe[:, 512 * a:512 * a + 512],
                                 start=(kt == 0), stop=(kt == NS - 1))

        # finalize: the pva/pvb->SBUF copies run immediately (freeing the
        # PSUM banks); the transpose/divide/stage steps become fillers
        # spread over the following segment's PE slack.
        def fin_item(seg, sb, sub, a):
            qt, m = seg["qt"], seg["m"]
            stage = stages[qt]
            trp = ps_st.tile([128, 128], F32, tag="st", name="trf")
            nc.tensor.transpose(trp[:, 0:65],
                                sb[0:65, 128 * sub:128 * sub + 128],
                                ident[0:65, 0:65])
            tsb = pvsbp.tile([128, 65], F32, tag="tsb", name="tsb")
            nc.vector.tensor_copy(tsb[:], trp[:, 0:65])
            r = recp.tile([128, 1], F32, tag="rec", name="r")
            dcol = 64 if a == 0 else 0
            vs = (0, 64) if a == 0 else (1, 65)
            nc.vector.reciprocal(r[:], tsb[:, dcol:dcol + 1])
            nc.vector.tensor_scalar_mul(
                stage[:, sub, 128 * m + 64 * a:128 * m + 64 * a + 64],
                tsb[:, vs[0]:vs[1]], r[:, 0:1])
            seg["fin_done"] = seg.get("fin_done", 0) + 1
            if seg["fin_done"] == 8 and m == NM - 1:
                for s2 in range(4):
                    nc.sync.dma_start(
                        out=out_d[512 * qt + 128 * s2:
                                  512 * qt + 128 * s2 + 128, :],
                        in_=stage[:, s2, :])

        flat = [(seg, kt) for seg in segs for kt in range(NG)]

        # pre-work: projections for the first segment's deps, then the
        # first two score groups so the exp stream starts 2 deep.
        proj_qk_nt("q", 0, 0)
        proj_qk_nt("k", 0, 0)
        pend = [emit_scores(*flat[0]), emit_scores(*flat[1])]

        for j, (seg, kt) in enumerate(flat):
            if j + 2 < len(flat):
                pend.append(emit_scores(*flat[j + 2]))
            pump((seg["idx"], kt))
            emit_pv(seg, kt, pend.pop(0))
            if kt == NG - 1:
                sba = pvsbp.tile([65, 512], F32, tag="pvsb", name="sba")
                nc.vector.tensor_copy(sba[:], seg["pva"][:])
                sbb = pvsbp.tile([65, 512], F32, tag="pvsb", name="sbb")
                nc.vector.tensor_copy(sbb[:], seg["pvb"][:])
                nidx = seg["idx"] + 1
                for sub in range(4):
                    for a in (0, 1):
                        sb = sba if a == 0 else sbb
                        fq.append(((nidx, 1 + 2 * sub + a),
                                   (lambda s_=seg, sb_=sb, su_=sub, a_=a:
                                    fin_item(s_, sb_, su_, a_))))
                fq.sort(key=lambda fd: fd[0])
        pump((99, 99))    # drain remaining fillers (last segment's finalize)

    nc.compile()
    return nc


def _get_nc():
    if "nc" not in _CACHE:
        _CACHE["nc"] = _build()
    return _CACHE["nc"]


def _prep(a, blocks, width):
    # [S, H] -> [blocks, 128, H/128, width] bf16, partition-major blocks
    import ml_dtypes
    a = np.asarray(a, dtype=np.float32).astype(ml_dtypes.bfloat16)
    a = a.reshape(blocks, width, H // 128, 128).transpose(0, 3, 2, 1)
    return np.ascontiguousarray(a)


def _run(inputs, trace=False, tmpdir=None):
    import ml_dtypes
    from concourse.bass_utils import run_bass_kernel_spmd

    nc = _get_nc()
    q, k, v = inputs["q"], inputs["k"], inputs["v"]
    wq, wk, wv = inputs["wq"], inputs["wk"], inputs["wv"]
    bq, bk, bv = inputs["bq"], inputs["bk"], inputs["bv"]
    NQ, NK, VW = 4, 8, 258

    def f32(a):
        return np.ascontiguousarray(np.asarray(a), dtype=np.float32)

    def wprep(a):
        # [H, 256] -> [128, 8, 256] bf16 (partition-major contraction blocks)
        a = np.asarray(a, dtype=np.float32).astype(ml_dtypes.bfloat16)
        return np.ascontiguousarray(a.reshape(NK, 128, -1).transpose(1, 0, 2))

    in_maps = []
    for c in range(CORES):
        b, g = divmod(c, CORES // B)
        sel = slice(GROUP_COLS * g, GROUP_COLS * g + GROUP_COLS)
        wvs = np.asarray(wv[:, sel], dtype=np.float32)   # [H, 256]
        wva = np.zeros((H, VW), dtype=np.float32)
        bva = np.zeros((1, VW), dtype=np.float32)
        bvs = f32(bv[sel])
        for m in range(2):
            wva[:, 129 * m:129 * m + 64] = wvs[:, 128 * m:128 * m + 64]
            wva[:, 129 * m + 65:129 * m + 129] = \
                wvs[:, 128 * m + 64:128 * m + 128]
            bva[0, 129 * m:129 * m + 64] = bvs[128 * m:128 * m + 64]
            bva[0, 129 * m + 64] = 1.0
            bva[0, 129 * m + 65:129 * m + 129] = \
                bvs[128 * m + 64:128 * m + 128]
        in_maps.append({
            "q": _prep(q[b], NQ, 512), "k": _prep(k[b], NQ, 512),
            "v": _prep(v[b], NQ, 512),
            "wq": wprep(wq[:, sel]), "wk": wprep(wk[:, sel]),
            "wv": np.ascontiguousarray(
                np.asarray(wva, dtype=np.float32).astype(ml_dtypes.bfloat16)
                .reshape(NK, 128, VW).transpose(1, 0, 2)),
            "bq": f32(bq[sel]).reshape(GROUP_COLS, 1),
            "bk": f32(bk[sel]).reshape(GROUP_COLS, 1),
            "bv": np.ascontiguousarray(bva.astype(ml_dtypes.bfloat16)),
        })

    res = run_bass_kernel_spmd(nc, in_maps, list(range(CORES)),
                               trace=trace, tmpdir=tmpdir)
    out = np.empty((B, S, H), dtype=np.float32)
    for c in range(CORES):
        b, g = divmod(c, CORES // B)
        out[b, :, GROUP_COLS * g:GROUP_COLS * g + GROUP_COLS] = \
            res.results[c]["out"]
    return out, res


def kernel(**inputs):
    out, _ = _run(inputs, trace=False)
    return out


# revision 7
# speedup vs baseline: 1.1695x; 1.1695x over previous
"""Multi-head attention (B=2, S=2048, H=1024, 16 heads x 64) on 8 trn2 cores.

Sharding: data-parallel over batch (2) x tensor-parallel over heads (4 groups
of 4 heads). Core c handles batch c//4, head-group c%4 (wq/wk/wv columns
[256*g, 256*g+256)). Host slices inputs per core (pre-cast to bf16 and
pre-blocked into the [quarter, partition, kb, seq] layout the SBUF tiles
use) and concatenates the per-core head-slice outputs.

Per-core schedule (bf16 matmuls, fp32 PSUM accumulation). The kernel is
ACT(exp)-bound in steady state, so everything is organized to start the
exp stream early and keep it dense:

  - inputs arrive as S/4 "quarter" blocks ([128, 8kb, 512] per tensor),
    DMA-prioritized so the first score tile's deps (wq, q0, wk, k0) land
    first; later quarters stream in behind the compute.
  - scores are computed transposed, ST[keys, q-512], as row-tiled pairs
    (two heads on PE row groups (0,0)/(64,0) run concurrently); one
    [128, 1024] PSUM tile per key-tile feeds a single 1024-col exp
    (scale=1/32; no max subtraction - logits are O(0.25) by construction).
    The score pipeline runs TWO groups ahead of the exp stream so filler
    chains never starve the ACT engine.
  - V is projected directly into the transposed [keys, d] layout the PV
    matmuls need: per key-tile, an 8-chunk chain with lhsT = v-chunk and
    rhs = wv (both head-pairs at once, FD=258), plus a ones-row x
    (bias|1|bias) rank-1 matmul that adds the bias AND writes the shared
    ones column - no PE transposes, no per-head copies.
  - PV accumulates out'^T [65, 512] over the 16 key tiles; the shared
    ones column gives the softmax denominator as row 64/0 for free.
  - finalize: PE-transpose out'^T back to [q, 65] (f32), reciprocal of
    the denominator, per-row scale, stage 4 heads, DMA out (f32).
  - projection/finalize work is drip-fed into the PE slack of the exp
    stream as deadline-tagged fillers (projection chains split into
    4-matmul halves so no single filler exceeds the exp-stream buffer);
    segments run m-major so the m=1 projections spread across the m=0
    segments.

The softmax mask of the reference is a mathematical no-op (it broadcasts
over the key axis, shifting every logit of a row equally), so it is ignored.
"""

import numpy as np

B, S, H = 2, 2048, 1024
NH, D = 16, 64            # heads, head_dim
CORES = 8
GROUP_COLS = 256          # 4 heads per core
SCALE = 1.0 / 32.0        # 1/sqrt(H)

_CACHE = {}


def _build():
    import concourse.bacc as bacc
    import concourse.tile as tile
    import concourse.mybir as mybir
    from concourse.masks import make_identity
    from contextlib import ExitStack

    F32 = mybir.dt.float32
    BF16 = mybir.dt.bfloat16
    EXP = mybir.ActivationFunctionType.Exp

    nc = bacc.Bacc("TRN2", target_bir_lowering=False, debug=False,
                   num_devices=CORES)

    NS = S // 128          # 16 key tiles
    NK = H // 128          # 8 contraction tiles over H
    NQ = S // 512          # 4 q-tiles / quarters of 512
    NM = 2                 # head-pairs per core
    VW = 2 * 129           # vh row: [A|1|B] per head-pair

    # blocked inputs: [quarter, partition, kb, seq-in-quarter]
    q_d = nc.dram_tensor("q", [NQ, 128, NK, 512], BF16,
                         kind="ExternalInput").ap()
    k_d = nc.dram_tensor("k", [NQ, 128, NK, 512], BF16,
                         kind="ExternalInput").ap()
    v_d = nc.dram_tensor("v", [NQ, 128, NK, 512], BF16,
                         kind="ExternalInput").ap()
    wq_d = nc.dram_tensor("wq", [128, NK, GROUP_COLS], BF16,
                          kind="ExternalInput").ap()
    wk_d = nc.dram_tensor("wk", [128, NK, GROUP_COLS], BF16,
                          kind="ExternalInput").ap()
    wv_d = nc.dram_tensor("wv", [128, NK, VW], BF16,
                          kind="ExternalInput").ap()
    bq_d = nc.dram_tensor("bq", [GROUP_COLS, 1], F32,
                          kind="ExternalInput").ap()
    bk_d = nc.dram_tensor("bk", [GROUP_COLS, 1], F32,
                          kind="ExternalInput").ap()
    bv_d = nc.dram_tensor("bv", [1, VW], BF16, kind="ExternalInput").ap()
    out_d = nc.dram_tensor("out", [S, GROUP_COLS], F32,
                           kind="ExternalOutput").ap()

    with tile.TileContext(nc) as tc, ExitStack() as es:
        const = es.enter_context(tc.tile_pool(name="const", bufs=1))
        xpool = es.enter_context(tc.tile_pool(name="x", bufs=1))
        wpool = es.enter_context(tc.tile_pool(name="w", bufs=1))
        proj = es.enter_context(tc.tile_pool(name="proj", bufs=1))
        vhp = es.enter_context(tc.tile_pool(name="vh", bufs=1))
        pexpp = es.enter_context(tc.tile_pool(name="pexp", bufs=8))
        pvsbp = es.enter_context(tc.tile_pool(name="pvsb", bufs=4))
        stagep = es.enter_context(tc.tile_pool(name="stage", bufs=1))
        recp = es.enter_context(tc.tile_pool(name="rec", bufs=8))
        # PSUM: st = [128,1024] x3 slots (6 banks; also serves projection
        # accumulators and finalize transposes); pva/pvb = 2 banks.
        ps_st = es.enter_context(tc.tile_pool(name="ps_st", bufs=3,
                                              space="PSUM"))
        ps_pv = es.enter_context(tc.tile_pool(name="ps_pv", bufs=1,
                                              space="PSUM"))

        ident = const.tile([128, 128], F32, tag="ident")
        make_identity(nc, ident[:])
        ones_row = const.tile([1, 128], BF16, tag="ones_row")
        nc.vector.memset(ones_row[:], 1.0)

        # ---- DMAs in priority order (SP issues in emission order) ----
        # v first so the VH chains can run in the otherwise-idle prefix
        # window; k/v quarters before the later q quarters (q[nt] is only
        # needed one segment ahead, k/v feed segment 0's tail).
        wvb = wpool.tile([128, NK, VW], BF16, tag="wvb")
        nc.sync.dma_start(out=wvb[:], in_=wv_d[:, :, :])
        bvt = const.tile([1, VW], BF16, tag="bvt")
        nc.sync.dma_start(out=bvt[:], in_=bv_d[:, :])
        xv = xpool.tile([128, NQ, NK, 512], BF16, tag="xv")
        nc.sync.dma_start(out=xv[:, 0], in_=v_d[0])
        wqb = wpool.tile([128, NK, GROUP_COLS], BF16, tag="wqb")
        nc.sync.dma_start(out=wqb[:], in_=wq_d[:, :, :])
        xq = xpool.tile([128, NQ, NK, 512], BF16, tag="xq")
        nc.sync.dma_start(out=xq[:, 0], in_=q_d[0])
        wkb = wpool.tile([128, NK, GROUP_COLS], BF16, tag="wkb")
        nc.sync.dma_start(out=wkb[:], in_=wk_d[:, :, :])
        xk = xpool.tile([128, NQ, NK, 512], BF16, tag="xk")
        nc.sync.dma_start(out=xk[:, 0], in_=k_d[0])
        bias_t = {}
        for x, b_d in (("q", bq_d), ("k", bk_d)):
            bt = const.tile([128, NM], F32, tag=f"b{x}", name=f"b{x}t")
            nc.sync.dma_start(
                out=bt[:], in_=b_d.rearrange("(m p) o -> p m o", p=128)
                .rearrange("p m o -> p (m o)"))
            for m in range(NM):
                bias_t[(x, m)] = bt[:, m:m + 1]
        for i in range(1, NQ):
            nc.sync.dma_start(out=xk[:, i], in_=k_d[i])
            nc.sync.dma_start(out=xv[:, i], in_=v_d[i])
        for i in range(1, NQ):
            nc.sync.dma_start(out=xq[:, i], in_=q_d[i])

        # persistent projection outputs
        QT = [proj.tile([128, S], BF16, tag=f"qt{m}", name=f"QT{m}")
              for m in range(NM)]
        KT = [proj.tile([128, S], BF16, tag=f"kt{m}", name=f"KT{m}")
              for m in range(NM)]
        VH = vhp.tile([128, NS, VW], BF16, tag="vh")

        wbf = {"q": wqb, "k": wkb}
        xbf = {"q": xq, "k": xk}

        def proj_qk_half(x, m, nt, half, st):
            # half 0: kb 0..3 into a fresh acc; half 1: kb 4..7 + evacuate.
            if half == 0:
                acc = ps_st.tile([128, 1024], F32, tag="st", name="acc")
                st["acc"] = acc
            acc = st["acc"]
            a = acc[:, 0:512]
            for kb in range(4 * half, 4 * half + 4):
                nc.tensor.matmul(
                    a, wbf[x][:, kb, 128 * m:128 * m + 128],
                    xbf[x][:, nt, kb, :],
                    start=(kb == 0), stop=(kb == NK - 1))
            if half == 1:
                dst = (QT if x == "q" else KT)[m][:, 512 * nt:512 * nt + 512]
                nc.vector.tensor_scalar_add(dst, a, bias_t[(x, m)])

        def proj_qk_nt(x, m, nt):
            st = {}
            proj_qk_half(x, m, nt, 0, st)
            proj_qk_half(x, m, nt, 1, st)

        def proj_vh(s):
            # VH[s] rows = [vh_A | 1 | vh_B] for key rows 128s..128s+128,
            # both head-pairs (cols [0,129) pair0, [129,258) pair1).
            nt, sub = divmod(s, 4)
            acc = ps_st.tile([128, 1024], F32, tag="st", name="acc")
            a = acc[:, 0:VW]
            for kb in range(NK):
                nc.tensor.matmul(
                    a, xv[:, nt, kb, 128 * sub:128 * sub + 128],
                    wvb[:, kb, :], start=(kb == 0), stop=False)
            # rank-1: adds bias everywhere and writes the ones column
            nc.tensor.matmul(a, ones_row[:, 0:128], bvt[:, :],
                             start=False, stop=True)
            nc.vector.tensor_copy(VH[:, s, :], a)

        stages = {qt: stagep.tile([128, 4, GROUP_COLS], F32, tag=f"stage{qt}",
                                  name=f"stage{qt}") for qt in range(NQ)}

        # ---- attention pipeline with deadline-driven PE fillers ----
        # segment = (m, qt); group = key tile kt (both heads, 1024 cols)
        NG = NS
        segs = [{"qt": qt, "m": m, "pva": None, "pvb": None, "idx": 4 * m + qt}
                for m in range(NM) for qt in range(NQ)]

        def half_filler(x, m, nt, dl):
            st = {}
            return [(dl, lambda: proj_qk_half(x, m, nt, 0, st)),
                    ((dl[0], dl[1] + 1), lambda: proj_qk_half(x, m, nt, 1, st))]

        fq = []
        for s in range(4, NS):                   # VH just-in-time in seg 0
            fq.append(((0, max(0, s - 2)), (lambda s_=s: proj_vh(s_))))
        fq += half_filler("k", 0, 1, (0, 0))
        fq += half_filler("k", 0, 2, (0, 4))
        fq += half_filler("k", 0, 3, (0, 8))
        fq += half_filler("q", 0, 1, (0, 12))    # QT[0] for seg 1
        fq += half_filler("k", 1, 0, (1, 3))
        fq += half_filler("k", 1, 1, (1, 8))
        fq += half_filler("q", 0, 2, (1, 12))    # QT[0] for seg 2
        fq += half_filler("k", 1, 2, (2, 3))
        fq += half_filler("k", 1, 3, (2, 8))
        fq += half_filler("q", 0, 3, (2, 12))    # QT[0] for seg 3
        fq += half_filler("q", 1, 0, (3, 7))     # QT[1] for seg 4
        fq += half_filler("q", 1, 1, (4, 12))    # QT[1] for seg 5
        fq += half_filler("q", 1, 2, (5, 12))
        fq += half_filler("q", 1, 3, (6, 12))
        fq.sort(key=lambda fd: fd[0])

        def pump(upto):
            while fq and fq[0][0] <= upto:
                fq.pop(0)[1]()

        def emit_scores(seg, kt):
            qt, m = seg["qt"], seg["m"]
            stt = ps_st.tile([128, 1024], F32, tag="st", name="stt")
            for a in (0, 1):
                p0 = 64 * a
                nc.tensor.matmul(
                    stt[:, 512 * a:512 * a + 512],
                    KT[m][p0:p0 + 64, 128 * kt:128 * kt + 128],
                    QT[m][p0:p0 + 64, 512 * qt:512 * qt + 512],
                    start=True, stop=True, tile_position=(p0, 0))
            pe = pexpp.tile([128, 1024], BF16, tag="pexp", name="pexp")
            nc.scalar.activation(pe[:], stt[:], EXP, scale=SCALE)
            return pe

        def emit_pv(seg, kt, pe):
            m = seg["m"]
            if seg["pva"] is None:
                seg["pva"] = ps_pv.tile([65, 512], F32, tag="pva", name="pva")
                seg["pvb"] = ps_pv.tile([65, 512], F32, tag="pvb", name="pvb")
            for a in (0, 1):
                pv = seg["pva"] if a == 0 else seg["pvb"]
                lo = 129 * m + 64 * a
                nc.tensor.matmul(pv[:], VH[:, kt, lo:lo + 65],
                                 pe[:, 512 * a:512 * a + 512],
                                 start=(kt == 0), stop=(kt == NS - 1))

        # finalize: the pva/pvb->SBUF copies run immediately (freeing the
        # PSUM banks); the transpose/divide/stage steps become fillers
        # spread over the following segment's PE slack.
        def fin_item(seg, sb, sub, a):
            qt, m = seg["qt"], seg["m"]
            stage = stages[qt]
            trp = ps_st.tile([128, 128], F32, tag="st", name="trf")
            nc.tensor.transpose(trp[:, 0:65],
                                sb[0:65, 128 * sub:128 * sub + 128],
                                ident[0:65, 0:65])
            tsb = pvsbp.tile([128, 65], F32, tag="tsb", name="tsb")
            nc.vector.tensor_copy(tsb[:], trp[:, 0:65])
            r = recp.tile([128, 1], F32, tag="rec", name="r")
            dcol = 64 if a == 0 else 0
            vs = (0, 64) if a == 0 else (1, 65)
            nc.vector.reciprocal(r[:], tsb[:, dcol:dcol + 1])
            nc.vector.tensor_scalar_mul(
                stage[:, sub, 128 * m + 64 * a:128 * m + 64 * a + 64],
                tsb[:, vs[0]:vs[1]], r[:, 0:1])
            seg["fin_done"] = seg.get("fin_done", 0) + 1
            if seg["fin_done"] == 8 and m == NM - 1:
                for s2 in range(4):
                    nc.sync.dma_start(
                        out=out_d[512 * qt + 128 * s2:
                                  512 * qt + 128 * s2 + 128, :],
                        in_=stage[:, s2, :])

        flat = [(seg, kt) for seg in segs for kt in range(NG)]

        # pre-work: VH[0..3] fills the v0-DMA-gated prefix window, then
        # projections for the first segment's scores.
        for s in range(4):
            proj_vh(s)
        proj_qk_nt("q", 0, 0)
        proj_qk_nt("k", 0, 0)
        pending = emit_scores(*flat[0])

        for j, (seg, kt) in enumerate(flat):
            if j + 1 < len(flat):
                nxt = emit_scores(*flat[j + 1])
            else:
                nxt = None
            pump((seg["idx"], kt))
            if kt < 2:
                # defer the segment's first two PVs past the boundary so a
                # late pva/pvb re-allocation never blocks the PE FIFO
                fq.append(((seg["idx"], kt + 2),
                           (lambda s_=seg, k_=kt, p_=pending: emit_pv(s_, k_, p_))))
                fq.sort(key=lambda fd: fd[0])
            else:
                emit_pv(seg, kt, pending)
            pending = nxt
            if kt == NG - 1:
                sba = pvsbp.tile([65, 512], F32, tag="pvsb", name="sba")
                nc.vector.tensor_copy(sba[:], seg["pva"][:])
                sbb = pvsbp.tile([65, 512], F32, tag="pvsb", name="sbb")
                nc.vector.tensor_copy(sbb[:], seg["pvb"][:])
                nidx = seg["idx"] + 1
                for sub in range(4):
                    for a in (0, 1):
                        sb = sba if a == 0 else sbb
                        fq.append(((nidx, 1 + 2 * sub + a),
                                   (lambda s_=seg, sb_=sb, su_=sub, a_=a:
                                    fin_item(s_, sb_, su_, a_))))
                fq.sort(key=lambda fd: fd[0])
        pump((99, 99))    # drain remaining fillers (last segment's finalize)

    nc.compile()
    return nc


def _get_nc():
    if "nc" not in _CACHE:
        _CACHE["nc"] = _build()
    return _CACHE["nc"]


def _prep(a, blocks, width):
    # [S, H] -> [blocks, 128, H/128, width] bf16, partition-major blocks
    import ml_dtypes
    a = np.asarray(a, dtype=np.float32).astype(ml_dtypes.bfloat16)
    a = a.reshape(blocks, width, H // 128, 128).transpose(0, 3, 2, 1)
    return np.ascontiguousarray(a)


def _run(inputs, trace=False, tmpdir=None):
    import ml_dtypes
    from concourse.bass_utils import run_bass_kernel_spmd

    nc = _get_nc()
    q, k, v = inputs["q"], inputs["k"], inputs["v"]
    wq, wk, wv = inputs["wq"], inputs["wk"], inputs["wv"]
    bq, bk, bv = inputs["bq"], inputs["bk"], inputs["bv"]
    NQ, NK, VW = 4, 8, 258

    def f32(a):
        return np.ascontiguousarray(np.asarray(a), dtype=np.float32)

    def wprep(a):
        # [H, 256] -> [128, 8, 256] bf16 (partition-major contraction blocks)
        a = np.asarray(a, dtype=np.float32).astype(ml_dtypes.bfloat16)
        return np.ascontiguousarray(a.reshape(NK, 128, -1).transpose(1, 0, 2))

    in_maps = []
    for c in range(CORES):
        b, g = divmod(c, CORES // B)
        sel = slice(GROUP_COLS * g, GROUP_COLS * g + GROUP_COLS)
        wvs = np.asarray(wv[:, sel], dtype=np.float32)   # [H, 256]
        wva = np.zeros((H, VW), dtype=np.float32)
        bva = np.zeros((1, VW), dtype=np.float32)
        bvs = f32(bv[sel])
        for m in range(2):
            wva[:, 129 * m:129 * m + 64] = wvs[:, 128 * m:128 * m + 64]
            wva[:, 129 * m + 65:129 * m + 129] = \
                wvs[:, 128 * m + 64:128 * m + 128]
            bva[0, 129 * m:129 * m + 64] = bvs[128 * m:128 * m + 64]
            bva[0, 129 * m + 64] = 1.0
            bva[0, 129 * m + 65:129 * m + 129] = \
                bvs[128 * m + 64:128 * m + 128]
        in_maps.append({
            "q": _prep(q[b], NQ, 512), "k": _prep(k[b], NQ, 512),
            "v": _prep(v[b], NQ, 512),
            "wq": wprep(wq[:, sel]), "wk": wprep(wk[:, sel]),
            "wv": np.ascontiguousarray(
                np.asarray(wva, dtype=np.float32).astype(ml_dtypes.bfloat16)
                .reshape(NK, 128, VW).transpose(1, 0, 2)),
            "bq": f32(bq[sel]).reshape(GROUP_COLS, 1),
            "bk": f32(bk[sel]).reshape(GROUP_COLS, 1),
            "bv": np.ascontiguousarray(bva.astype(ml_dtypes.bfloat16)),
        })

    res = run_bass_kernel_spmd(nc, in_maps, list(range(CORES)),
                               trace=trace, tmpdir=tmpdir)
    out = np.empty((B, S, H), dtype=np.float32)
    for c in range(CORES):
        b, g = divmod(c, CORES // B)
        out[b, :, GROUP_COLS * g:GROUP_COLS * g + GROUP_COLS] = \
            res.results[c]["out"]
    return out, res


def kernel(**inputs):
    out, _ = _run(inputs, trace=False)
    return out
